# revision 91
# baseline (speedup 1.0000x reference)
"""Causal self-attention on 8 trn2 NeuronCores.

Sharding: core c = (b, g) with b = c // 4 (batch), g = c % 4 (head group of
4 heads).  Each core computes q/k/v projections for its 4 heads, causal
attention, and a partial out-projection (its 256 rows of Wout).  Host sums
the 4 partials per batch and adds bout.

Layouts on device (x / qkv weights ship as bf16 to halve startup DMA;
attention operands are f32r — both run at 1 PE cycle/row, psum is f32):
  xT   [128, 8, 2048]   x[b] transposed, d-tile major      (lhsT/rhs source)
  qT/kT[128, 2, 2048]   features on partitions, s free
  vaug [128, 16, 4, 65] s-tile major, per head 64 v-cols + ones col
  attn [128, 2, 2048]   normalized attention output, f on partitions
Scores are computed as sT[k, q] so the attn@V matmul needs no transposes;
softmax denominator comes from the ones column of vaug; exp() is
unnormalized (scores are O(1) by construction, no overflow risk).
"""

import sys

if "/opt/trn_rl_repo" not in sys.path:
    sys.path.insert(0, "/opt/trn_rl_repo")

import numpy as np

import concourse.mybir as mybir
import concourse.tile as tile
from concourse import bacc
from concourse.bass_utils import run_bass_kernel_spmd
from concourse.vector_clock import ScopedClock, VectorClock

B, S, D, H, HD = 2, 2048, 1024, 16, 64
G = 4            # head groups (cores per batch)
HL = H // G      # heads per core = 4
FL = HL * HD     # local features = 256
NQB = S // 512   # 4 q-blocks of 512
NST = S // 128   # 16 s-tiles of 128
NDT = D // 128   # 8 d-tiles

F32 = mybir.dt.float32
F32R = mybir.dt.float32r
BF16 = mybir.dt.bfloat16
EXPF = mybir.ActivationFunctionType.Exp


class SplitDrainTC(tile.TileContext):
    """This walrus build rejects >1 sync wait on an SP Drain; emit one
    drain per live proc instead of a single fat one."""

    def _drain_and_barrier(self, tick_clock, wait_clock):
        g = tick_clock.global_clock
        n = len(g)
        live = [(p, g[p]) for p in range(n) if g[p] > 0]
        if not live:
            self.nc.sync.drain()
        for p, t in live:
            vec = [0] * n
            vec[p] = t
            d = self.nc.sync.drain()
            wait_clock.add_sem_waits(d.ins, ScopedClock({None: VectorClock(vec)}))
        self.nc.all_engine_barrier()
        assert self.sems is not None
        popped = self.nc._tile_sem_poison_stack.pop()
        assert popped is self._sem_poison
        self.nc.clear_and_free_semaphores(list(self.sems.allocated().values()))
        self.nc.all_engine_barrier()


def _build(debug=False):
    nc = bacc.Bacc()
    xT = nc.declare_dram_parameter("xT", [128, NDT, S], BF16, isOutput=False)
    wq = nc.declare_dram_parameter("wq", [128, NDT, FL], BF16, isOutput=False)
    wk = nc.declare_dram_parameter("wk", [128, NDT, FL], BF16, isOutput=False)
    wv = nc.declare_dram_parameter("wv", [128, NDT, FL], BF16, isOutput=False)
    wout = nc.declare_dram_parameter("wout", [128, 2, D], F32R, isOutput=False)
    tri = nc.declare_dram_parameter("tri", [128, 128], F32R, isOutput=False)
    out_p = nc.declare_dram_parameter("out_p", [S, D], BF16, isOutput=True)
    if debug:
        qT_d = nc.declare_dram_parameter("qT_d", [128, 2, S], F32, isOutput=True)
        kT_d = nc.declare_dram_parameter("kT_d", [128, 2, S], F32, isOutput=True)
        vaug_d = nc.declare_dram_parameter(
            "vaug_d", [128, NST, HL, HD + 1], F32, isOutput=True
        )
        attn_d = nc.declare_dram_parameter("attn_d", [128, 2, S], F32, isOutput=True)
        exp_d = nc.declare_dram_parameter("exp_d", [16, 128, 1024], F32, isOutput=True)

    from collections import deque
    from contextlib import ExitStack

    with SplitDrainTC(nc) as tc, ExitStack() as ctx:
        consts = ctx.enter_context(tc.tile_pool(name="consts", bufs=1))
        pp_big = ctx.enter_context(tc.tile_pool(name="pp_big", bufs=2, space="PSUM"))
        pp_fill = ctx.enter_context(tc.tile_pool(name="pp_fill", bufs=2, space="PSUM"))
        pp_attn = ctx.enter_context(tc.tile_pool(name="pp_attn", bufs=2, space="PSUM"))
        pool_exp = ctx.enter_context(tc.tile_pool(name="pool_exp", bufs=5))
        pool_out = ctx.enter_context(tc.tile_pool(name="pool_out", bufs=5))
        pool_sm = ctx.enter_context(tc.tile_pool(name="pool_sm", bufs=4))

        xT_sb = consts.tile([128, NDT, S], BF16)
        wq_sb = consts.tile([128, NDT, FL], BF16)
        wk_sb = consts.tile([128, NDT, FL], BF16)
        wv_sb = consts.tile([128, NDT, FL], BF16)
        wout_sb = consts.tile([128, 2, D], F32R)
        tri_sb = consts.tile([128, 128], F32R)
        qT_sb = consts.tile([128, 2, S], F32R)
        kT_sb = consts.tile([128, 2, S], F32R)
        vaug_sb = consts.tile([128, NST, HL, HD + 1], F32R)
        attn_sb = consts.tile([128, 2, S], F32R)

        # ACT spline-table preload for Exp overlaps the initial DMAs
        warm = pool_sm.tile([1, 1], F32, tag="warm")
        nc.vector.memset(warm, 0.0)
        nc.scalar.activation(out=warm, in_=warm, func=EXPF)

        # PE clock-ramp warmup: ~3.5us of dummy matmuls on zeroed SBUF while
        # the first DMAs land, so real matmuls start at full clock.
        nc.vector.memset(attn_sb[:, 0, 0:256].bitcast(F32), 0.0)
        for i in range(24):
            wps = pp_fill.tile([128, 512], F32, tag="fill")
            nc.tensor.matmul(
                wps[:, 0:128],
                attn_sb[:, 0, 0:128],
                attn_sb[:, 0, 128:256],
                start=True,
                stop=True,
            )

        # DMA order matters: first matmuls need wq/wk and the first s-block
        # of xT; wv before the prologue v-chunks; wout only at first out-proj.
        # weights issue from the (idle-at-start) ACT queue so their
        # descriptor generation runs parallel to the xT stream on SP;
        # SP sequencer DMA issue costs ~0.65us each, so consolidate.
        nc.scalar.dma_start(out=wq_sb, in_=wq[:])
        for t in range(NDT):
            nc.sync.dma_start(out=xT_sb[:, t, 0:512], in_=xT[:, t, 0:512])
        nc.scalar.dma_start(out=wk_sb, in_=wk[:])
        nc.scalar.dma_start(out=wv_sb, in_=wv[:])
        nc.scalar.dma_start(out=tri_sb, in_=tri[:])
        for t in range(NDT):
            nc.sync.dma_start(
                out=xT_sb[:, t, 512:S], in_=xT[:, t, 512:S]
            )
        nc.scalar.dma_start(out=wout_sb, in_=wout[:])
        # ones columns of vaug (constant across the run)
        nc.gpsimd.memset(vaug_sb[:, :, :, HD : HD + 1].bitcast(F32), 1.0)

        # ---- chunk emitters (projections / out-proj used as PE filler) ----
        def qkT_chunk(w_sb, dst, ft, sb_):
            def emit():
                ps = pp_fill.tile([128, 512], F32, tag="fill")
                for dt_ in range(NDT):
                    nc.tensor.matmul(
                        ps[:, 0:512],
                        w_sb[:, dt_, ft * 128 : ft * 128 + 128],
                        xT_sb[:, dt_, sb_ * 512 : sb_ * 512 + 512],
                        start=(dt_ == 0),
                        stop=(dt_ == NDT - 1),
                    )
                nc.vector.tensor_copy(
                    out=dst[:, ft, sb_ * 512 : sb_ * 512 + 512], in_=ps[:, 0:512]
                )

            return emit

        def v_chunk(st):
            def emit():
                ps = pp_fill.tile([128, 512], F32, tag="fill")
                for dt_ in range(NDT):
                    nc.tensor.matmul(
                        ps[:, 0:FL],
                        xT_sb[:, dt_, st * 128 : st * 128 + 128],
                        wv_sb[:, dt_, :],
                        start=(dt_ == 0),
                        stop=(dt_ == NDT - 1),
                    )
                nc.vector.tensor_copy(
                    out=vaug_sb[:, st, :, 0:HD],
                    in_=ps[:, 0:FL].rearrange("p (h e) -> p h e", h=HL),
                )

            return emit

        def oproj_chunk(q0, late=False):
            def emit():
                for dc in range(2):
                    ops = pp_fill.tile([128, 512], F32, tag="fill")
                    for ft in range(2):
                        nc.tensor.matmul(
                            ops[:, 0:512],
                            attn_sb[:, ft, q0 : q0 + 128],
                            wout_sb[:, ft, dc * 512 : dc * 512 + 512],
                            start=(ft == 0),
                            stop=(ft == 1),
                        )
                    out_t = pool_out.tile([128, 512], BF16, tag="out")
                    if late:
                        # the kernel tail is DVE-bound; ACT is idle there
                        nc.scalar.copy(out=out_t, in_=ops[:, 0:512])
                    else:
                        nc.vector.tensor_copy(out=out_t, in_=ops[:, 0:512])
                    nc.sync.dma_start(
                        out=out_p[q0 : q0 + 128, dc * 512 : dc * 512 + 512], in_=out_t
                    )

            return emit

        # filler queue: (deadline_qb, cost_ns, emit_fn); FIFO order respects deps
        queue = deque()
        reserve = deque()
        for qb in range(1, NQB):
            for w_sb, dst in ((wq_sb, qT_sb), (wk_sb, kT_sb)):
                for ft in range(2):
                    queue.append((qb, 1750, qkT_chunk(w_sb, dst, ft, qb)))
            for st in range(4 * qb, 4 * qb + 4):
                queue.append((qb, 900, v_chunk(st)))

        # Adaptive pump: spread remaining filler cost over remaining attention
        # steps so late q-blocks (which have no projections left) still get
        # out-proj chunks as PE filler.
        total_steps = sum(2 * (4 * qb + 4) for qb in range(NQB))  # 80
        # out-proj chunks enter the queue late; count their cost up front
        future_oproj = 4 * NQB * 900
        step_no = 0

        def pump():
            nonlocal step_no, future_oproj
            step_no += 1
            remaining = sum(c for _, c, _ in queue) + future_oproj
            budget = remaining / max(total_steps - step_no, 1)
            while queue and budget >= queue[0][1] * 1.3:
                _, cost, emit = queue.popleft()
                emit()
                budget -= cost

        def drain_due(qb):
            while queue and queue[0][0] <= qb:
                _, _, emit = queue.popleft()
                emit()

        # ---- prologue: everything attention qb=0 needs ----
        for w_sb, dst in ((wq_sb, qT_sb), (wk_sb, kT_sb)):
            for ft in range(2):
                qkT_chunk(w_sb, dst, ft, 0)()
        for st in range(4):
            v_chunk(st)()

        # ---- attention (scores -> exp/mask -> lagged attnV), PE kept dense ----
        for qb in range(NQB):
            drain_due(qb)
            for pair in range(2):
                ha, hb = 2 * pair, 2 * pair + 1
                aps_a = pp_attn.tile([128, 512], F32, tag="acc")
                aps_b = pp_attn.tile([128, 512], F32, tag="acc")
                nkb = 4 * qb + 4
                lagged = deque()  # expt tiles awaiting their attnV matmuls

                def attnv(
                    expt, kb, soff, nkb=nkb, aps_a=aps_a, aps_b=aps_b, ha=ha, hb=hb
                ):
                    nc.tensor.matmul(
                        aps_a[0:65, soff:512],
                        vaug_sb[:, kb, ha, :],
                        expt[:, soff:512],
                        start=(kb == 0),
                        stop=(kb == nkb - 1),
                    )
                    nc.tensor.matmul(
                        aps_b[0:65, soff:512],
                        vaug_sb[:, kb, hb, :],
                        expt[:, 512 + soff : 1024],
                        start=(kb == 0),
                        stop=(kb == nkb - 1),
                    )

                for kb in range(nkb):
                    r = kb - 4 * qb
                    # causally-dead q columns are skipped when the remaining
                    # width keeps f32r at full rate (>=256)
                    soff = {1: 128, 2: 256, 3: 256}.get(r, 0)
                    sps = pp_big.tile([128, 1024], F32, tag="ps")
                    # scores^T [k, q]; two heads on disjoint row groups
                    nc.tensor.matmul(
                        sps[:, soff:512],
                        kT_sb[0:64, pair, kb * 128 : kb * 128 + 128],
                        qT_sb[0:64, pair, qb * 512 + soff : qb * 512 + 512],
                        start=True,
                        stop=True,
                    )
                    nc.tensor.matmul(
                        sps[:, 512 + soff : 1024],
                        kT_sb[64:128, pair, kb * 128 : kb * 128 + 128],
                        qT_sb[64:128, pair, qb * 512 + soff : qb * 512 + 512],
                        start=True,
                        stop=True,
                    )
                    expt = pool_exp.tile([128, 1024], F32R, tag="expt")
                    if r <= 0:
                        nc.scalar.activation(out=expt, in_=sps, func=EXPF, scale=0.125)
                        if r == 0:
                            tri_eng = (
                                nc.gpsimd
                                if qb == NQB - 1 and pair == 1
                                else nc.vector
                            )
                            for half in (0, 512):
                                tri_eng.tensor_mul(
                                    expt[:, half : half + 128],
                                    expt[:, half : half + 128],
                                    tri_sb,
                                )
                    else:
                        off = 128 * r
                        # one 2D-AP exp covers both heads' valid strips
                        nc.scalar.activation(
                            out=expt.rearrange("p (h q) -> p h q", h=2)[
                                :, :, off:512
                            ],
                            in_=sps.rearrange("p (h q) -> p h q", h=2)[:, :, off:512],
                            func=EXPF,
                            scale=0.125,
                        )
                        for half in (0, 512):
                            if soff < off:
                                # attnV reads [soff:512]; zero the dead strip
                                nc.gpsimd.memset(
                                    expt[:, half + soff : half + off].bitcast(F32),
                                    0.0,
                                )
                            tri_eng = (
                                nc.gpsimd
                                if qb == NQB - 1 and pair == 1
                                else nc.vector
                            )
                            tri_eng.tensor_mul(
                                expt[:, half + off : half + off + 128],
                                expt[:, half + off : half + off + 128],
                                tri_sb,
                            )
                    if debug and qb == 1 and pair == 0:
                        nc.sync.dma_start(out=exp_d[kb, :, :], in_=expt.bitcast(F32))
                    lagged.append((expt, kb, soff))
                    if len(lagged) > 4:
                        attnv(*lagged.popleft())
                    pump()
                while lagged:
                    attnv(*lagged.popleft())
                last = qb == NQB - 1 and pair == 1
                # evacuate accumulators promptly (frees the psum banks for the
                # next pair); normalization happens off the critical path
                for h, aps in ((ha, aps_a), (hb, aps_b)):
                    au = pool_sm.tile([128, 512], F32, tag="au")
                    nc.scalar.copy(out=au[0:65, :], in_=aps[0:65, :])
                    rec = pool_sm.tile([1, 512], F32, tag="rec")
                    nc.vector.reciprocal(out=rec, in_=au[64:65, :])
                    brec = pool_sm.tile([64, 512], F32, tag="brec")
                    nc.gpsimd.partition_broadcast(brec, rec)
                    mul_eng = (
                        nc.gpsimd if (qb == NQB - 1 and pair == 0) else nc.vector
                    )
                    mul_eng.tensor_mul(
                        attn_sb[
                            64 * (h % 2) : 64 * (h % 2) + 64,
                            pair,
                            qb * 512 : qb * 512 + 512,
                        ],
                        au[0:64, :],
                        brec,
                    )
                if last:
                    # reserved out-proj chunks fill PE while the final
                    # normalization chains run
                    while reserve:
                        reserve.popleft()()

            if debug and qb == NQB - 1:
                nc.sync.dma_start(out=qT_d[:], in_=qT_sb.bitcast(F32))
                nc.sync.dma_start(out=kT_d[:], in_=kT_sb.bitcast(F32))
                nc.sync.dma_start(out=vaug_d[:], in_=vaug_sb.bitcast(F32))
                nc.sync.dma_start(out=attn_d[:], in_=attn_sb.bitcast(F32))

            # out-projection for this q-block becomes future PE filler;
            # hold back two late chunks for the very end of attention
            for qs in range(4):
                late = qb == NQB - 1 or (qb == NQB - 2 and qs >= 2)
                ch = oproj_chunk(qb * 512 + qs * 128, late=late)
                if qb >= NQB - 2 and qs >= 2:
                    reserve.append(ch)
                else:
                    queue.append((NQB + 1, 900, ch))
                future_oproj -= 900

        while reserve:
            reserve.popleft()()
        while queue:
            _, _, emit = queue.popleft()
            emit()

    nc.compile()
    return nc


_NC = None


def _get_nc():
    global _NC
    if _NC is None:
        _NC = _build()
    return _NC


def kernel(x, mask, Wqkv, bqkv, Wout, bout):
    x = np.asarray(x, dtype=np.float32)
    Wqkv = np.asarray(Wqkv, dtype=np.float32)
    bqkv = np.asarray(bqkv, dtype=np.float32)
    Wout = np.asarray(Wout, dtype=np.float32)
    bout = np.asarray(bout, dtype=np.float32)
    assert not np.any(bqkv), "nonzero bqkv not supported by this kernel"

    import ml_dtypes

    bf16 = ml_dtypes.bfloat16

    # host-side layout prep; x and the qkv weights ship as bf16 (halves the
    # startup DMA; projections accumulate in f32 psum)
    xTs = []
    for b in range(B):
        xt = np.ascontiguousarray(
            x[b].T.reshape(NDT, 128, S).transpose(1, 0, 2).astype(bf16)
        )  # [128, 8, 2048]
        xTs.append(xt)
    tri = np.ascontiguousarray(np.triu(np.ones((128, 128), dtype=np.float32)))

    def wslice(j, g):  # j: 0=q,1=k,2=v
        cols = Wqkv[:, j * D + g * FL : j * D + (g + 1) * FL]  # [1024, 256]
        return np.ascontiguousarray(
            cols.reshape(NDT, 128, FL).transpose(1, 0, 2).astype(bf16)
        )

    in_maps = []
    for c in range(8):
        b, g = c // G, c % G
        wo = Wout[g * FL : (g + 1) * FL, :]  # [256, 1024]
        in_maps.append(
            {
                "xT": xTs[b],
                "wq": wslice(0, g),
                "wk": wslice(1, g),
                "wv": wslice(2, g),
                "wout": np.ascontiguousarray(wo.reshape(2, 128, D).transpose(1, 0, 2)),
                "tri": tri,
            }
        )

    nc = _get_nc()
    # axon terminals occasionally flake (transient NRT_EXEC_UNIT errors);
    # a retry of the same dispatch succeeds
    import time as _time

    res = None
    for attempt in range(3):
        try:
            res = run_bass_kernel_spmd(nc, in_maps, core_ids=list(range(8)))
            break
        except Exception:
            if attempt == 2:
                raise
            _time.sleep(2.0)

    out = np.empty((B, S, D), dtype=np.float32)
    for b in range(B):
        acc = res.results[b * G]["out_p"].astype(np.float32).copy()
        for g in range(1, G):
            acc += res.results[b * G + g]["out_p"]
        out[b] = acc + bout[None, :]
    return out
